# revision 2
# baseline (speedup 1.0000x reference)
"""Multi-head self-attention Trainium2 kernel v6, sharded over 8 NeuronCores.

Sharding: core = (batch, head_group): 2 batches x 4 head-groups (4 heads each).
Each core: qkv for its (batch, heads), full-seq attention, row-parallel slice
of the output projection -> partial [T, C] fp16. Host sums partials + b_proj.

Structure:
- x uploaded bf16, x^T loaded via DMA-transpose (XBAR); no PE transposes.
- weights uploaded host-reshaped to [128, N] SBUF images -> one DMA each.
- scores: [128,1024] psum tiles on a bufs=3 ring, exp on ACT -> fp16 pt
  tiles; AV as out[q,65] with ones-column denominators accumulated in a
  shared 1-bank psum region (6 rotating column slots).
- epilogue: per-partition reciprocal + tensor_scalar_mul -> o_pair fp16;
  O^T via SBUF->SBUF DMA transpose; proj fp16 -> fp16 partial out.
- emission: fine-grained interleave — between consecutive score tiles PE
  gets at most ~900ns of other work (one AV group or one filler item), so
  the ACT exp stream never starves; qk/v/proj run on the separate "mm"
  psum tag so the score ring parity is never disturbed.
"""

import math
import sys

import ml_dtypes
import numpy as np

sys.path.insert(0, "/opt/trn_rl_repo")

import concourse.bacc as bacc
import concourse.tile as tile
from concourse import mybir
from concourse.bass_utils import run_bass_kernel_spmd

B, T, C = 2, 2048, 1024
NH, DH = 16, 64
HG = 4                 # heads per core
DL = HG * DH           # 256 local qk channels
DV = HG * (DH + 1)     # 260: v columns + one ones-column per head
N_CORES = 8

F32 = mybir.dt.float32
BF16 = mybir.dt.bfloat16
F16 = mybir.dt.float16

SCALE = 1.0 / math.sqrt(DH)

N_AV_SLOTS = 6         # av-bank column slots (65 cols each, 390 <= 512)
N_SC = 8               # score tiles per unit, [128,1024] = 2 st-chunks each


def build_bass():
    nc = bacc.Bacc("TRN2", target_bir_lowering=False, debug=False)

    x_bt = nc.declare_dram_parameter("x_bt", [T, C], BF16, isOutput=False)
    w_qkm = nc.declare_dram_parameter("w_qkm", [128, 2 * 8 * 256], BF16,
                                      isOutput=False)
    w_v = nc.declare_dram_parameter("w_v", [128, 8 * DV], BF16, isOutput=False)
    biases = nc.declare_dram_parameter("biases", [128, 4 + DV], F32,
                                       isOutput=False)
    w_p = nc.declare_dram_parameter("w_p", [128, 2 * C], F16, isOutput=False)
    out = nc.declare_dram_parameter("out_partial", [T, C], F16, isOutput=True)

    Exp = mybir.ActivationFunctionType.Exp

    with tile.TileContext(nc) as tc:
        with (
            tc.tile_pool(name="singles", bufs=1) as singles,
            tc.tile_pool(name="pt", bufs=1) as ptp,
            tc.tile_pool(name="opair", bufs=8) as opp,
            tc.tile_pool(name="osmall", bufs=6) as osmall,
            tc.tile_pool(name="oout", bufs=3) as ooutp,
            tc.tile_pool(name="ps", bufs=1, space="PSUM") as psp,
        ):
            # ---- DMAs: ordered so K(m2, tb0) is executable earliest -------
            bias_sb = singles.tile([128, 4 + DV], F32)
            nc.sync.dma_start(out=bias_sb[:], in_=biases[:])

            wqk_all = singles.tile([128, 2 * 8 * 256], BF16, name="wqk_all")
            nc.sync.dma_start(out=wqk_all[:], in_=w_qkm[:])

            xt = [singles.tile([128, T], BF16, name=f"xt{ci}") for ci in range(8)]

            def xt_batch(tb):
                for ci in range(8):
                    nc.sync.dma_start_transpose(
                        out=xt[ci][:, tb * 512:(tb + 1) * 512],
                        in_=x_bt[tb * 512:(tb + 1) * 512,
                                 ci * 128:(ci + 1) * 128])

            xt_batch(0)
            xt_batch(1)
            xt_batch(2)
            xt_batch(3)
            wv_all = singles.tile([128, 8 * DV], BF16, name="wv_all")
            nc.sync.dma_start(out=wv_all[:], in_=w_v[:])
            wp_all = singles.tile([128, 2 * C], F16, name="wp_all")
            nc.sync.dma_start(out=wp_all[:], in_=w_p[:])

            # persistent SBUF results
            qkt = [singles.tile([128, T], F16, name=f"qkt{m}") for m in range(4)]
            v_all = singles.tile([128, 16 * DV], F16, name="v_all")
            ot = [singles.tile([128, T], F16, name=f"ot{di}") for di in range(2)]
            av_bank = psp.tile([128, 512], F32, tag="av", name="av_bank")

            def qk_half(sel, m, tb):
                """One qk projection chunk [128,512] on the mm psum tag."""
                base = 2048 if sel == "q" else 0
                lm = m % 2
                ps = psp.tile([128, 512], F32, tag="mm", name=f"qk{m}_{tb}")
                for ci in range(8):
                    nc.tensor.matmul(
                        ps[:],
                        lhsT=wqk_all[:, base + ci * 256 + lm * 128:
                                     base + ci * 256 + (lm + 1) * 128],
                        rhs=xt[ci][:, tb * 512:(tb + 1) * 512],
                        start=(ci == 0),
                        stop=(ci == 7),
                    )
                nc.vector.tensor_scalar_add(
                    qkt[m][:, tb * 512:(tb + 1) * 512],
                    ps[:], bias_sb[:, m:m + 1])

            def v_block(tt):
                ps = psp.tile([128, DV], F32, tag="mm", name=f"vps{tt}")
                for ci in range(8):
                    nc.tensor.matmul(
                        ps[:],
                        lhsT=xt[ci][:, tt * 128:(tt + 1) * 128],
                        rhs=wv_all[:, ci * DV:(ci + 1) * DV],
                        start=(ci == 0),
                        stop=(ci == 7),
                    )
                nc.vector.tensor_add(
                    v_all[:, tt * DV:(tt + 1) * DV], ps[:],
                    bias_sb[:, 4:4 + DV])

            live = {}

            def proj_half(tt, nb, tag="mm"):
                key = ("oout", tt)
                if key not in live:
                    live[key] = ooutp.tile([128, C], F16, tag="oout",
                                           name=f"oout{tt}")
                o_out = live[key]
                ps = psp.tile([128, 512], F32, tag=tag,
                              bufs=3 if tag == "sc" else 1,
                              name=f"pr{tt}_{nb}")
                for di in range(2):
                    nc.tensor.matmul(
                        ps[:],
                        lhsT=ot[di][:, tt * 128:(tt + 1) * 128],
                        rhs=wp_all[:, di * C + nb * 512:di * C + (nb + 1) * 512],
                        start=(di == 0),
                        stop=(di == 1),
                    )
                nc.vector.tensor_copy(o_out[:, nb * 512:(nb + 1) * 512], ps[:])
                if nb == 1:
                    nc.sync.dma_start(
                        out=out[tt * 128:(tt + 1) * 128, :], in_=o_out[:])
                    del live[key]

            av_slot_ctr = [0]

            class Unit:
                def __init__(self, h, qb):
                    self.h, self.qb = h, qb
                    self.moff = (h % 2) * 64
                    self.q_tile = qkt[h // 2]
                    self.k_tile = qkt[2 + h // 2]
                    self.pts = []
                    self.sc_i = 0

                def sc_tile(self):
                    k = self.sc_i
                    self.sc_i += 1
                    ps = psp.tile([128, 1024], F32, tag="sc", bufs=3,
                                  name=f"sc{self.h}_{self.qb}_{k}")
                    for half in range(2):
                        st = 2 * k + half
                        nc.tensor.matmul(
                            ps[:, half * 512:(half + 1) * 512],
                            lhsT=self.k_tile[self.moff:self.moff + 64,
                                             st * 128:(st + 1) * 128],
                            rhs=self.q_tile[self.moff:self.moff + 64,
                                            self.qb * 512:(self.qb + 1) * 512],
                            start=True,
                            stop=True,
                        )
                    pt = ptp.tile([128, 1024], F16, tag="pt", bufs=34,
                                  name=f"pt{self.h}_{self.qb}_{k}")
                    nc.scalar.activation(pt[:], ps[:], Exp, scale=SCALE)
                    self.pts.append(pt)

                def av_group(self, j):
                    h, qb = self.h, self.qb
                    qt = qb * 4 + j
                    slot = av_slot_ctr[0] % N_AV_SLOTS
                    av_slot_ctr[0] += 1
                    co = slot * 65
                    for p in range(N_SC):
                        for half in range(2):
                            st = 2 * p + half
                            nc.tensor.matmul(
                                av_bank[:, co:co + 65],
                                lhsT=self.pts[p][:, half * 512 + j * 128:
                                                 half * 512 + (j + 1) * 128],
                                rhs=v_all[:, st * DV + h * (DH + 1):
                                          st * DV + (h + 1) * (DH + 1)],
                                start=(st == 0),
                                stop=(st == 15),
                            )
                    rec = osmall.tile([128, 1], F32, tag="rec")
                    nc.vector.reciprocal(rec[:], av_bank[:, co + 64:co + 65])
                    pair = h // 2
                    key = ("op", pair, qt)
                    if key not in live:
                        live[key] = opp.tile([128, 128], F16, tag="op",
                                             name=f"op{pair}_{qt}")
                    op_t = live[key]
                    nc.vector.tensor_scalar_mul(
                        op_t[:, (h % 2) * 64:(h % 2) * 64 + 64],
                        av_bank[:, co:co + 64], rec[:])
                    if h % 2 == 1:
                        nc.sync.dma_start_transpose(
                            out=ot[pair][:, qt * 128:(qt + 1) * 128],
                            in_=op_t[:])
                        del live[key]

            # ---- emission -------------------------------------------------
            # Per-unit ordered work lists, popped one item per inter-tile
            # gap. AV groups start at unit 3 (all V blocks must land first:
            # v0-7 in u1, v8-15 in u2), double up in u3/u4 to reach lag-1.
            units = [Unit(i % 4, i // 4) for i in range(16)]

            def avf(uu, j):
                return lambda: units[uu].av_group(j)

            def pjf(tt, nb, tag="mm"):
                return lambda: proj_half(tt, nb, tag)

            def vf(tt):
                return lambda: v_block(tt)

            def qf(sel, m, tb):
                return lambda: qk_half(sel, m, tb)

            work = {}
            late = {}
            work[4] = [f for j in range(4) for f in (avf(2, j), avf(3, j))]
            late[4] = [qf("q", 1, 1)]
            work[5] = [avf(4, 0), pjf(0, 0), avf(4, 1), pjf(0, 1),
                       avf(4, 2), pjf(1, 0), avf(4, 3)]
            work[6] = [avf(5, 0), pjf(1, 1), avf(5, 1), pjf(2, 0),
                       avf(5, 2), pjf(2, 1), avf(5, 3)]
            work[7] = [avf(6, 0), qf("q", 0, 2), avf(6, 1), pjf(3, 0),
                       avf(6, 2), pjf(3, 1), avf(6, 3)]
            work[8] = [avf(7, 0), qf("q", 1, 2), avf(7, 1), avf(7, 2),
                       avf(7, 3)]
            work[9] = [avf(8, 0), pjf(4, 0), avf(8, 1), pjf(4, 1),
                       avf(8, 2), pjf(5, 0), avf(8, 3)]
            work[10] = [avf(9, 0), pjf(5, 1), avf(9, 1), pjf(6, 0),
                        avf(9, 2), pjf(6, 1), avf(9, 3)]
            work[11] = [avf(10, 0), qf("q", 0, 3), avf(10, 1), pjf(7, 0),
                        avf(10, 2), pjf(7, 1), avf(10, 3)]
            work[12] = [avf(11, 0), qf("q", 1, 3), avf(11, 1), avf(11, 2),
                        avf(11, 3)]
            work[13] = [avf(12, 0), pjf(8, 0), avf(12, 1), pjf(8, 1),
                        avf(12, 2), pjf(9, 0), avf(12, 3)]
            work[14] = [avf(13, 0), pjf(9, 1), avf(13, 1), pjf(10, 0),
                        avf(13, 2), pjf(10, 1), avf(13, 3)]
            work[15] = [avf(14, 0), pjf(11, 0), avf(14, 1), pjf(11, 1),
                        avf(14, 2), avf(14, 3)]
            late = {**{i: [] for i in range(4, 16)}, **late}
            late[4] = [qf("q", 1, 1)]

            # ---- unit 0: staggered K halves (only m2 gates unit 0);
            # its last two tiles (needing K tb3) are deferred into unit 1
            u0 = units[0]
            qk_half("k", 2, 0)
            qk_half("q", 0, 0)
            u0.sc_tile()              # st 0-1
            qk_half("k", 2, 1)
            u0.sc_tile()              # st 2-3
            qk_half("k", 2, 2)
            u0.sc_tile()              # st 4-5
            u0.sc_tile()              # st 6-7
            u0.sc_tile()              # st 8-9
            u0.sc_tile()              # st 10-11

            # ---- unit 1: finish u0 (K tb3), own tiles with v0-7 ----------
            u1 = units[1]
            qk_half("k", 2, 3)
            u0.sc_tile()              # st 12-13
            u0.sc_tile()              # st 14-15
            for k in range(N_SC):
                u1.sc_tile()
                v_block(k)            # v0..v7
            qk_half("q", 1, 0)
            qk_half("k", 3, 0)

            # ---- unit 2 (h2): K m3 halves interleaved + v8-15 -------------
            u2 = units[2]
            u2.sc_tile()
            v_block(8)
            u2.sc_tile()
            qk_half("k", 3, 1)
            u2.sc_tile()
            v_block(9)
            u2.sc_tile()
            qk_half("k", 3, 2)
            u2.sc_tile()
            v_block(10)
            u2.sc_tile()
            qk_half("k", 3, 3)
            u2.sc_tile()
            v_block(11)
            u2.sc_tile()
            v_block(12)
            v_block(13)
            v_block(14)
            v_block(15)

            # ---- unit 3: AV catch-up (u0 + u1) ---------------------------
            u3 = units[3]
            u3.sc_tile()
            units[0].av_group(0)
            u3.sc_tile()
            units[0].av_group(1)
            u3.sc_tile()
            units[0].av_group(2)
            u3.sc_tile()
            units[0].av_group(3)
            u3.sc_tile()
            units[1].av_group(0)
            u3.sc_tile()
            units[1].av_group(1)
            u3.sc_tile()
            units[1].av_group(2)
            u3.sc_tile()
            units[1].av_group(3)
            qk_half("q", 0, 1)

            for i in range(4, 16):
                u = units[i]
                wl = list(work[i])
                for k in range(N_SC):
                    u.sc_tile()
                    if wl:
                        wl.pop(0)()
                for f in wl + late[i]:
                    f()

            # ---- tail: last unit's AV + final projections -----------------
            last = units[15]
            last.av_group(0)
            last.av_group(1)
            proj_half(12, 0, "sc")
            last.av_group(2)
            proj_half(12, 1, "mm")
            last.av_group(3)
            proj_half(13, 0, "sc")
            proj_half(13, 1, "mm")
            proj_half(14, 0, "sc")
            proj_half(14, 1, "mm")
            proj_half(15, 0, "sc")
            proj_half(15, 1, "mm")

    nc.compile()
    return nc


_CACHE = {}


def _get_nc():
    if "nc" not in _CACHE:
        _CACHE["nc"] = build_bass()
    return _CACHE["nc"]


def make_in_maps(x, w_qkv, b_qkv, w_proj):
    in_maps = []
    for core in range(N_CORES):
        b = core // 4
        hg = core % 4
        cs = slice(hg * DL, (hg + 1) * DL)
        wq = w_qkv[:, 0 * C:1 * C][:, cs]      # [C, 256]
        wk = w_qkv[:, 1 * C:2 * C][:, cs]
        wv = w_qkv[:, 2 * C:3 * C][:, cs]
        bq = b_qkv[0 * C:1 * C][cs]
        bk = b_qkv[1 * C:2 * C][cs]
        bv = b_qkv[2 * C:3 * C][cs]
        # v extended: per head 64 v-cols + a ones column (softmax denominator)
        w_v_ext = np.zeros((C, DV), dtype=np.float32)
        b_v_ext = np.zeros((DV,), dtype=np.float32)
        for hh in range(HG):
            w_v_ext[:, hh * (DH + 1):hh * (DH + 1) + DH] = \
                wv[:, hh * DH:(hh + 1) * DH]
            b_v_ext[hh * (DH + 1):hh * (DH + 1) + DH] = \
                bv[hh * DH:(hh + 1) * DH]
            b_v_ext[hh * (DH + 1) + DH] = 1.0

        # host-reshaped [128, 8*N] images: col block ci holds rows ci*128+p
        def img(w):  # [C, N] -> [128, 8*N]
            return np.concatenate(
                [w[ci * 128:(ci + 1) * 128, :] for ci in range(8)], axis=1)

        wp_img = np.concatenate(
            [w_proj[cs, :][di * 128:(di + 1) * 128, :] for di in range(2)],
            axis=1)
        in_maps.append({
            "x_bt": np.ascontiguousarray(x[b]).astype(ml_dtypes.bfloat16),
            "w_qkm": np.ascontiguousarray(
                np.concatenate([img(wk), img(wq)], axis=1)).astype(
                    ml_dtypes.bfloat16),
            "w_v": np.ascontiguousarray(img(w_v_ext)).astype(ml_dtypes.bfloat16),
            "biases": np.concatenate(
                [np.stack([np.concatenate([bq, bk])[m * 128:(m + 1) * 128]
                           for m in range(4)], axis=1),
                 np.broadcast_to(b_v_ext[None, :], (128, DV))],
                axis=1).astype(np.float32),
            "w_p": np.ascontiguousarray(wp_img).astype(np.float16),
        })
    return in_maps


def kernel(x, w_qkv, b_qkv, w_proj, b_proj, **runner_kwargs):
    x = np.asarray(x, dtype=np.float32)
    w_qkv = np.asarray(w_qkv, dtype=np.float32)
    b_qkv = np.asarray(b_qkv, dtype=np.float32)
    w_proj = np.asarray(w_proj, dtype=np.float32)
    b_proj = np.asarray(b_proj, dtype=np.float32)

    nc = _get_nc()
    in_maps = make_in_maps(x, w_qkv, b_qkv, w_proj)
    res = run_bass_kernel_spmd(nc, in_maps, list(range(N_CORES)), **runner_kwargs)
    parts = [np.asarray(res.results[i]["out_partial"], dtype=np.float32)
             for i in range(N_CORES)]
    outv = np.zeros((B, T, C), dtype=np.float32)
    for b in range(B):
        outv[b] = parts[4 * b + 0] + parts[4 * b + 1] \
            + parts[4 * b + 2] + parts[4 * b + 3]
        outv[b] += b_proj[None, :]
    if runner_kwargs:
        return outv, res
    return outv


if __name__ == "__main__":
    import reference

    inputs = reference.setup_inputs()
    inputs = {k: np.asarray(v) for k, v in inputs.items()}
    got = kernel(**inputs)
    want = np.asarray(reference.reference(**inputs))
    err = np.abs(got - want).max() / np.abs(want).max()
    print("rel err:", err)
